# revision 5
# baseline (speedup 1.0000x reference)
"""ConvCNP encoder kernel for 8x TRN2 NeuronCores.

Math: the reference computes, for a 128x128 uniform grid g=(xs[i], ys[j]) and
n=8192 data points X (2-D) with values psi(Y) = [1, Y0, Y1]:

    Gram[g, x] = exp(-0.5*||g - X[x]||^2)
    fm = Gram @ psi                  # (G, 3); column 0 == row-sum (denominator)
    out[c, j, i] = fm[(i, j), c], with c=1,2 normalized by column 0.

The squared distance is separable over the grid axes:

    Gram[(i,j), x] = A[i, x] * B[j, x]
      A[i, x] = exp(-0.5*(xs[i] - X0[x])^2)     B[j, x] = exp(-0.5*(ys[j] - X1[x])^2)

so, with Bc = B * psi_c (row-wise):  fm[(i,j), c] = sum_x Bc[j, x] * A[i, x].

Low-rank grid compression: A[i, x] varies analytically in the *grid*
coordinate xs[i], so its 128 rows are interpolated from NANCH=12 anchor rows

    P[k, x] = exp(-0.5*(a_k - X0[x])^2),   A ~= F^T P
    (F = fixed [12, 128] ridge-LS interpolation weights, host precomputed;
     max |F| ~ 1, approximation error ~3e-6, final error fp16-dominated)

which cuts the A-side elementwise work ~10x and shrinks the per-chunk matmul
weight loads from 128 to 12 columns:

    M[k, (c,j)]   = sum_x P[k, x] * BfT[x, (c,j)]   (64 PE chunks, PSUM accum)
    acc[i, (c,j)] = sum_k F[k, i] * M[k, (c,j)]     (one tiny fp32 matmul)

Layout: BfT is stored [x_part, (c,j), chunk] (chunk innermost) so the
B*psi broadcast multiplies have packed innermost dims on every operand,
which enables the DVE 2x fp16 mode; the matmul rhs reads the 48 (c,j)
columns of chunk k with a stride-NCHUNK access pattern.

Sharding: grid y-axis (j) across the 8 cores - 16 j-rows per core; X, Y
replicated. No cross-core communication. Inputs land via 3 parallel DMA
queues (sync/tensor/gpsimd) so the elementwise front starts ~0.7us earlier.
"""

import numpy as np
from contextlib import ExitStack

N_AXIS = 128          # grid points per axis
NPTS = 8192           # data points
NCORES = 8
JS = N_AXIS // NCORES  # 16 grid-y rows per core
NCHUNK = NPTS // 128   # 64 contraction chunks of 128
NANCH = 12             # anchor rows interpolating the 128 grid-x rows
ANCH_PAD = 0.3         # anchors span [-2-pad, 2+pad] at Chebyshev nodes
GRID_LO, GRID_HI = -2.0, 2.0
B_STRIPES = (4, 12, 24, 24)   # B-side chunk stripes (front-loaded ramp)
P_STRIPES = (16, 48)          # P-side chunk stripes

_CACHE = {}


def _anchors_and_weights():
    """Anchor positions a[NANCH] and interpolation weights F[NANCH, 128]
    (host constants; ridge least-squares fit of the 128 grid-row Gaussians
    in the anchor-Gaussian basis over a dense data-coordinate grid)."""
    xs = np.linspace(GRID_LO, GRID_HI, N_AXIS)
    half = (GRID_HI - GRID_LO) / 2 + ANCH_PAD
    t = np.cos((2 * np.arange(NANCH) + 1) / (2 * NANCH) * np.pi)
    a = t * half
    xd = np.linspace(-4.5, 4.5, 2001)
    Pd = np.exp(-0.5 * (a[:, None] - xd[None, :]) ** 2)
    Td = np.exp(-0.5 * (xs[:, None] - xd[None, :]) ** 2)
    G = Pd @ Pd.T
    lam = 1e-10 * np.trace(G) / NANCH
    F = np.linalg.solve(G + lam * np.eye(NANCH), Pd @ Td.T)  # (NANCH, 128)
    return a.astype(np.float32), F.astype(np.float32)


def _register_sqdiff():
    """Register a fused (a-b)^2 custom DVE op (idempotent)."""
    from concourse import dve_ops
    from concourse.dve_spec import Spec, Src0, Src1, sq, lower
    from concourse.dve_uop import DveOpSpec

    name = "TENSOR_SQDIFF_X"
    for op in dve_ops.OPS:
        if op.name == name:
            return op
    spec = Spec(
        body=sq(Src0 - Src1),
        reference=lambda in0, in1, s0, s1, imm2: (in0.astype(np.float32) - in1) ** 2,
    )
    opcode = max(dve_ops._SUB_OPCODE_FOR_NAME.values()) + 1
    assert opcode < 0x20
    dve_ops._SUB_OPCODE_FOR_NAME[name] = opcode
    shas = {}
    for ver in ("v3", "v4"):
        s = DveOpSpec(name=name, opcode=opcode, uops=lower(spec, ver=ver), rd1_en=True)
        shas[ver] = s.sha(ver)
    op = dve_ops.DveOp(name, spec, subdim=False, uops_sha=shas)
    dve_ops.OPS.append(op)
    dve_ops.CUSTOM_DVE_SPECS[name] = spec
    return op


def _patch_walrus_flags():
    """Cap the compiler's semaphore file (idempotent)."""
    import concourse.bass_utils as bu

    if getattr(bu.run_command, "_sem_cap_patched", False):
        return
    orig = bu.run_command

    def run_command_capped(argv, **kwargs):
        if argv and "walrus_driver" in str(argv[0]) and any(
                str(a).startswith("--neff-output-filename") for a in argv):
            argv = list(argv) + ["--max-sem-num=176"]
        return orig(argv, **kwargs)

    run_command_capped._sem_cap_patched = True
    bu.run_command = run_command_capped


def _build_program():
    import concourse.bacc as bacc
    import concourse.mybir as mybir
    import concourse.tile as tile

    _patch_walrus_flags()
    sqdiff = _register_sqdiff()

    f32 = mybir.dt.float32
    f16 = mybir.dt.float16
    nc = bacc.Bacc("TRN2", target_bir_lowering=False, debug=False, num_devices=NCORES,
                   enable_partition_id=False, monotonic_sem_count=0)

    # Packed inputs, one per DMA queue so they land in parallel:
    #   bc [128, 80]  f32: x1t(0:64) | ysb(64:80)      -> B chain (per-core ysb)
    #   ac [128, 76]  f32: x0t(0:64) | asb(64:76)      -> P anchor chain
    #   yc [128, 128] f16: y0t(0:64) | y1t(64:128)     -> B*psi muls
    #   fc [12, 128]  f32: F interpolation weights     -> decompression matmul
    bc = nc.dram_tensor("bc", [128, 80], f32, kind="ExternalInput")
    ac = nc.dram_tensor("ac", [128, 64 + NANCH], f32, kind="ExternalInput")
    yc = nc.dram_tensor("yc", [128, 128], f16, kind="ExternalInput")
    fc = nc.dram_tensor("fc", [NANCH, 128], f32, kind="ExternalInput")
    out = nc.dram_tensor("out", [128, 3 * JS], f32, kind="ExternalOutput")

    with tile.TileContext(nc) as tc, ExitStack() as ctx:
        singles = ctx.enter_context(tc.tile_pool(name="singles", bufs=1))
        psum = ctx.enter_context(tc.tile_pool(name="psum", bufs=1, space="PSUM"))

        s_bc = singles.tile([128, 80], f32, tag="bc")
        nc.sync.dma_start(s_bc[:, :], bc[:, :])
        s_ac = singles.tile([128, 64 + NANCH], f32, tag="ac")
        nc.gpsimd.dma_start(s_ac[:, :], ac[:, :])
        s_yc = singles.tile([128, 128], f16, tag="yc")
        nc.gpsimd.dma_start(s_yc[:, :], yc[:, :])
        s_fc = singles.tile([NANCH, 128], f32, tag="fc")
        nc.gpsimd.dma_start(s_fc[:, :], fc[:, :])

        x1t = s_bc[:, 0:64]
        ysb = s_bc[:, 64:80]
        x0t = s_ac[:, 0:64]
        asb = s_ac[:, 64:64 + NANCH]

        # Layouts: P [x_p, chunk, k] (k innermost = contiguous LDWEIGHTS);
        # B factors [x_p, (c,j), chunk] (chunk innermost = 2x DVE muls).
        s_psq = singles.tile([128, NCHUNK, NANCH], f32, tag="psq")
        s_p = singles.tile([128, NCHUNK, NANCH], f16, tag="p")
        s_bsq = singles.tile([128, JS, NCHUNK], f32, tag="bsq")
        s_bf = singles.tile([128, 3 * JS, NCHUNK], f16, tag="bf")
        yv = s_yc[:, :].rearrange("p (c k) -> p c k", c=2)

        # Interleaved producer chains. DVE order: bsq0, psq_a, bsq1, mul0,
        # psq_b, mul1, bsq2, mul2, bsq3, mul3. ACT order: bexp0, pexp_a,
        # bexp1, pexp_b, bexp2, bexp3. The matmul chain needs pexp_a+mul0
        # for chunk 0 and stays one stripe behind the muls.
        p_done = 0
        b_done = 0

        def psq_stripe(width):
            nonlocal p_done
            ks = slice(p_done, p_done + width)
            nc.vector._custom_dve(
                sqdiff,
                out=s_psq[:, ks, :],
                in0=asb.unsqueeze(1).broadcast_to([128, width, NANCH]),
                in1=x0t[:, ks].unsqueeze(2).broadcast_to([128, width, NANCH]),
            )
            p_done += width
            return ks

        def bsq_stripe(width):
            nonlocal b_done
            ks = slice(b_done, b_done + width)
            nc.vector._custom_dve(
                sqdiff,
                out=s_bsq[:, :, ks],
                in0=ysb.unsqueeze(2).broadcast_to([128, JS, width]),
                in1=x1t[:, ks].unsqueeze(1).broadcast_to([128, JS, width]),
            )
            b_done += width
            return ks

        def pexp_stripe(ks):
            nc.scalar.activation(
                s_p[:, ks, :], s_psq[:, ks, :],
                mybir.ActivationFunctionType.Exp, scale=-0.5,
            )

        def bexp_stripe(ks):
            nc.scalar.activation(
                s_bf[:, 0:JS, ks], s_bsq[:, :, ks],
                mybir.ActivationFunctionType.Exp, scale=-0.5,
            )

        def mul_stripe(ks):
            w = ks.stop - ks.start
            # [B*Y0 | B*Y1] in one 2x-mode op: every operand fp16 with
            # packed innermost (chunk) dim.
            nc.vector.tensor_tensor(
                s_bf[:, JS:3 * JS, ks].rearrange("p (c j) k -> p c j k", c=2),
                s_bf[:, 0:JS, ks].unsqueeze(1).broadcast_to([128, 2, JS, w]),
                yv[:, :, ks].unsqueeze(2).broadcast_to([128, 2, JS, w]),
                mybir.AluOpType.mult,
            )

        bs0 = bsq_stripe(B_STRIPES[0])
        ps_a = psq_stripe(P_STRIPES[0])
        bexp_stripe(bs0)
        bs1 = bsq_stripe(B_STRIPES[1])
        pexp_stripe(ps_a)
        mul_stripe(bs0)
        ps_b = psq_stripe(P_STRIPES[1])
        bexp_stripe(bs1)
        mul_stripe(bs1)
        bs2 = bsq_stripe(B_STRIPES[2])
        pexp_stripe(ps_b)
        bexp_stripe(bs2)
        mul_stripe(bs2)
        bs3 = bsq_stripe(B_STRIPES[3])
        bexp_stripe(bs3)
        mul_stripe(bs3)
        assert p_done == NCHUNK and b_done == NCHUNK

        # ---- contraction: M[k, (c,j)] = sum_x P[k,x] * BfT[x,(c,j)] ----
        accm = psum.tile([NANCH, 3 * JS], f32, tag="accm")
        for k in range(NCHUNK):
            nc.tensor.matmul(
                accm[:, :],
                s_p[:, k, :],        # stationary lhsT: [128, 12] fp16
                s_bf[:, :, k],       # moving rhs: [128, 48] fp16 (strided)
                start=(k == 0),
                stop=(k == NCHUNK - 1),
            )

        # ---- decompress: acc[i, (c,j)] = sum_k F[k,i] * M[k,(c,j)] ----
        s_m = singles.tile([NANCH, 3 * JS], f32, tag="m")
        nc.vector.tensor_copy(s_m[:, :], accm[:, :])
        acc = psum.tile([128, 3 * JS], f32, tag="acc")
        nc.tensor.matmul(
            acc[:, :],
            s_fc[:, :],              # stationary lhsT: [12, 128] fp32
            s_m[:, :],               # moving rhs: [12, 48] fp32
            start=True,
            stop=True,
        )

        # ---- epilogue: normalize columns 1,2 by column 0 (the row-sum) ----
        s_rec = singles.tile([128, JS], f32, tag="rec")
        nc.vector.reciprocal_approx_fast(s_rec[:, :], acc[:, 0:JS])
        s_out = singles.tile([128, 3 * JS], f32, tag="outt")
        nc.scalar.copy(s_out[:, 0:JS], acc[:, 0:JS])
        nc.vector.tensor_tensor(
            s_out[:, JS:3 * JS].rearrange("p (c j) -> p c j", c=2),
            acc[:, JS:3 * JS].rearrange("p (c j) -> p c j", c=2),
            s_rec[:, :].unsqueeze(1).broadcast_to([128, 2, JS]),
            mybir.AluOpType.mult,
        )
        nc.sync.dma_start(out[:, :], s_out[:, :])

    nc.finalize()
    return nc


def _get_program():
    if "nc" not in _CACHE:
        _CACHE["nc"] = _build_program()
    return _CACHE["nc"]


def _host_inputs(X, Y):
    """Build the per-core input maps (layout prep + host constants only)."""
    X = np.ascontiguousarray(np.asarray(X, dtype=np.float32))
    Y = np.ascontiguousarray(np.asarray(Y, dtype=np.float32))
    ys = np.linspace(GRID_LO, GRID_HI, N_AXIS, dtype=np.float32)
    a, F = _anchors_and_weights()

    ac = np.empty((128, 64 + NANCH), np.float32)
    ac[:, 0:64] = X[:, 0].reshape(NCHUNK, 128).T
    ac[:, 64:64 + NANCH] = a[None, :]
    yc = np.empty((128, 128), np.float16)
    yc[:, 0:64] = Y[:, 0].reshape(NCHUNK, 128).T
    yc[:, 64:128] = Y[:, 1].reshape(NCHUNK, 128).T

    x1t = X[:, 1].reshape(NCHUNK, 128).T
    in_maps = []
    for m in range(NCORES):
        bcm = np.empty((128, 80), np.float32)
        bcm[:, 0:64] = x1t
        bcm[:, 64:80] = ys[m * JS:(m + 1) * JS][None, :]
        in_maps.append({"bc": bcm, "ac": ac, "yc": yc, "fc": F})
    return in_maps


def run_on_cores(X, Y, **spmd_kwargs):
    """Run the SPMD kernel; returns BassKernelResults."""
    from concourse.bass_utils import run_bass_kernel_spmd

    nc = _get_program()
    in_maps = _host_inputs(X, Y)
    res = run_bass_kernel_spmd(nc, in_maps, core_ids=list(range(NCORES)),
                               **spmd_kwargs)
    return res


def kernel(X, Y):
    res = run_on_cores(X, Y)
    full = np.empty((3, N_AXIS, N_AXIS), dtype=np.float32)
    for m, r in enumerate(res.results):
        blk = r["out"]  # [128, 48] rows = i, cols = (c, j_local)
        for c in range(3):
            full[c, m * JS:(m + 1) * JS, :] = blk[:, c * JS:(c + 1) * JS].T
    return full


# revision 6
# speedup vs baseline: 1.1929x; 1.1929x over previous
"""ConvCNP encoder kernel for 8x TRN2 NeuronCores.

Math: the reference computes, for a 128x128 uniform grid g=(xs[i], ys[j]) and
n=8192 data points X (2-D) with values psi(Y) = [1, Y0, Y1]:

    Gram[g, x] = exp(-0.5*||g - X[x]||^2)
    fm = Gram @ psi                  # (G, 3); column 0 == row-sum (denominator)
    out[c, j, i] = fm[(i, j), c], with c=1,2 normalized by column 0.

The squared distance is separable over the grid axes:

    Gram[(i,j), x] = A[i, x] * B[j, x]
      A[i, x] = exp(-0.5*(xs[i] - X0[x])^2)     B[j, x] = exp(-0.5*(ys[j] - X1[x])^2)

so, with Bc = B * psi_c (row-wise):  fm[(i,j), c] = sum_x Bc[j, x] * A[i, x].

Low-rank grid compression: A[i, x] varies analytically in the *grid*
coordinate xs[i], so its 128 rows are interpolated from NANCH=12 anchor rows

    P[k, x] = exp(-0.5*(a_k - X0[x])^2),   A ~= F^T P
    (F = fixed [12, 128] ridge-LS interpolation weights, host precomputed;
     max |F| ~ 1, approximation error ~3e-6, final error fp16-dominated)

which cuts the A-side elementwise work ~10x and shrinks the per-chunk matmul
weight loads from 128 to 12 columns:

    M[k, (c,j)]   = sum_x P[k, x] * BfT[x, (c,j)]   (64 PE chunks, PSUM accum)
    acc[i, (c,j)] = sum_k F[k, i] * M[k, (c,j)]     (one fp16 matmul)

BfT keeps the chunk-contiguous [x_p, chunk, (c,j)] layout (the matmul moving
operand must read 48 contiguous columns — a strided rhs halves PE cadence).
The B*psi broadcast multiplies still hit the DVE 2x fp16 mode because the
Y operand arrives pre-broadcast from the host (yr[x_p, chunk, c, j]), making
every mul operand packed-innermost.

Sharding: grid y-axis (j) across the 8 cores - 16 j-rows per core; X, Y
replicated. No cross-core communication. Inputs ride 2 parallel DMA queues.
"""

import numpy as np
from contextlib import ExitStack

N_AXIS = 128          # grid points per axis
NPTS = 8192           # data points
NCORES = 8
JS = N_AXIS // NCORES  # 16 grid-y rows per core
NCHUNK = NPTS // 128   # 64 contraction chunks of 128
NANCH = 12             # anchor rows interpolating the 128 grid-x rows
ANCH_PAD = 0.3         # anchors span [-2-pad, 2+pad] at Chebyshev nodes
GRID_LO, GRID_HI = -2.0, 2.0
B_STRIPES = (4, 12, 24, 24)   # B-side chunk stripes (front-loaded ramp)
P_STRIPES = (16, 48)          # P-side chunk stripes

_CACHE = {}


def _anchors_and_weights():
    """Anchor positions a[NANCH] and interpolation weights F[NANCH, 128]
    (host constants; ridge least-squares fit of the 128 grid-row Gaussians
    in the anchor-Gaussian basis over a dense data-coordinate grid)."""
    xs = np.linspace(GRID_LO, GRID_HI, N_AXIS)
    half = (GRID_HI - GRID_LO) / 2 + ANCH_PAD
    t = np.cos((2 * np.arange(NANCH) + 1) / (2 * NANCH) * np.pi)
    a = t * half
    xd = np.linspace(-4.5, 4.5, 2001)
    Pd = np.exp(-0.5 * (a[:, None] - xd[None, :]) ** 2)
    Td = np.exp(-0.5 * (xs[:, None] - xd[None, :]) ** 2)
    G = Pd @ Pd.T
    lam = 1e-10 * np.trace(G) / NANCH
    F = np.linalg.solve(G + lam * np.eye(NANCH), Pd @ Td.T)  # (NANCH, 128)
    return a.astype(np.float32), F.astype(np.float32)


def _register_sqdiff():
    """Register a fused (a-b)^2 custom DVE op (idempotent)."""
    from concourse import dve_ops
    from concourse.dve_spec import Spec, Src0, Src1, sq, lower
    from concourse.dve_uop import DveOpSpec

    name = "TENSOR_SQDIFF_X"
    for op in dve_ops.OPS:
        if op.name == name:
            return op
    spec = Spec(
        body=sq(Src0 - Src1),
        reference=lambda in0, in1, s0, s1, imm2: (in0.astype(np.float32) - in1) ** 2,
    )
    opcode = max(dve_ops._SUB_OPCODE_FOR_NAME.values()) + 1
    assert opcode < 0x20
    dve_ops._SUB_OPCODE_FOR_NAME[name] = opcode
    shas = {}
    for ver in ("v3", "v4"):
        s = DveOpSpec(name=name, opcode=opcode, uops=lower(spec, ver=ver), rd1_en=True)
        shas[ver] = s.sha(ver)
    op = dve_ops.DveOp(name, spec, subdim=False, uops_sha=shas)
    dve_ops.OPS.append(op)
    dve_ops.CUSTOM_DVE_SPECS[name] = spec
    return op


def _patch_walrus_flags():
    """Cap the compiler's semaphore file (idempotent)."""
    import concourse.bass_utils as bu

    if getattr(bu.run_command, "_sem_cap_patched", False):
        return
    orig = bu.run_command

    def run_command_capped(argv, **kwargs):
        if argv and "walrus_driver" in str(argv[0]) and any(
                str(a).startswith("--neff-output-filename") for a in argv):
            argv = list(argv) + ["--max-sem-num=176"]
        return orig(argv, **kwargs)

    run_command_capped._sem_cap_patched = True
    bu.run_command = run_command_capped


def _build_program():
    import concourse.bacc as bacc
    import concourse.mybir as mybir
    import concourse.tile as tile

    _patch_walrus_flags()
    sqdiff = _register_sqdiff()

    f32 = mybir.dt.float32
    f16 = mybir.dt.float16
    nc = bacc.Bacc("TRN2", target_bir_lowering=False, debug=False, num_devices=NCORES,
                   enable_partition_id=False, monotonic_sem_count=0)

    # Packed inputs across two parallel DMA queues (issue order):
    #   sync:   bc [128, 80] f32: x1t | ysb        -> B chain (per-core ysb)
    #           yr [128, 64*32] f16: Y broadcast   -> B*psi muls (pre-broadcast)
    #   gpsimd: ac [128, 76] f32: x0t | asb        -> P anchor chain
    #           fc [12, 128] f16: F weights        -> decompression matmul
    bc = nc.dram_tensor("bc", [128, 80], f32, kind="ExternalInput")
    yr = nc.dram_tensor("yr", [128, NCHUNK * 2 * JS], f16, kind="ExternalInput")
    ac = nc.dram_tensor("ac", [128, 64 + NANCH], f32, kind="ExternalInput")
    fc = nc.dram_tensor("fc", [NANCH, 128], f16, kind="ExternalInput")
    out = nc.dram_tensor("out", [128, 3 * JS], f32, kind="ExternalOutput")

    with tile.TileContext(nc) as tc, ExitStack() as ctx:
        singles = ctx.enter_context(tc.tile_pool(name="singles", bufs=1))
        psum = ctx.enter_context(tc.tile_pool(name="psum", bufs=1, space="PSUM"))

        s_bc = singles.tile([128, 80], f32, tag="bc")
        nc.sync.dma_start(s_bc[:, :], bc[:, :])
        s_ac = singles.tile([128, 64 + NANCH], f32, tag="ac")
        nc.gpsimd.dma_start(s_ac[:, :], ac[:, :])
        s_yr = singles.tile([128, NCHUNK, 2, JS], f16, tag="yr")
        nc.sync.dma_start(
            s_yr[:, :, :, :].rearrange("p a b c -> p (a b c)"), yr[:, :])
        s_fc = singles.tile([NANCH, 128], f16, tag="fc")
        nc.gpsimd.dma_start(s_fc[:, :], fc[:, :])

        x1t = s_bc[:, 0:64]
        ysb = s_bc[:, 64:80]
        x0t = s_ac[:, 0:64]
        asb = s_ac[:, 64:64 + NANCH]

        # P [x_p, chunk, k] (k innermost -> contiguous LDWEIGHTS);
        # B factors [x_p, chunk, (c,j)] (chunk-major -> contiguous matmul rhs).
        s_psq = singles.tile([128, NCHUNK, NANCH], f32, tag="psq")
        s_p = singles.tile([128, NCHUNK, NANCH], f16, tag="p")
        s_bsq = singles.tile([128, NCHUNK, JS], f32, tag="bsq")
        s_bf = singles.tile([128, NCHUNK, 3 * JS], f16, tag="bf")

        # Interleaved producer chains. DVE order: bsq0, psq_a, bsq1, mul0,
        # psq_b, mul1, bsq2, mul2, bsq3, mul3. ACT order: bexp0, pexp_a,
        # bexp1, pexp_b, bexp2, bexp3. The matmul chain needs pexp_a+mul0
        # for chunk 0 and stays one stripe behind the muls.
        p_done = 0
        b_done = 0

        def psq_stripe(width):
            nonlocal p_done
            ks = slice(p_done, p_done + width)
            nc.vector._custom_dve(
                sqdiff,
                out=s_psq[:, ks, :],
                in0=asb.unsqueeze(1).broadcast_to([128, width, NANCH]),
                in1=x0t[:, ks].unsqueeze(2).broadcast_to([128, width, NANCH]),
            )
            p_done += width
            return ks

        def bsq_stripe(width):
            nonlocal b_done
            ks = slice(b_done, b_done + width)
            nc.vector._custom_dve(
                sqdiff,
                out=s_bsq[:, ks, :],
                in0=ysb.unsqueeze(1).broadcast_to([128, width, JS]),
                in1=x1t[:, ks].unsqueeze(2).broadcast_to([128, width, JS]),
            )
            b_done += width
            return ks

        def pexp_stripe(ks):
            nc.scalar.activation(
                s_p[:, ks, :], s_psq[:, ks, :],
                mybir.ActivationFunctionType.Exp, scale=-0.5,
            )

        def bexp_stripe(ks):
            nc.scalar.activation(
                s_bf[:, ks, 0:JS], s_bsq[:, ks, :],
                mybir.ActivationFunctionType.Exp, scale=-0.5,
            )

        def mul_stripe(ks):
            w = ks.stop - ks.start
            # [B*Y0 | B*Y1] in one 2x-mode op: every operand fp16 with a
            # packed innermost dim (yr arrives pre-broadcast over j).
            nc.vector.tensor_tensor(
                s_bf[:, ks, JS:3 * JS].rearrange("p k (c j) -> p k c j", c=2),
                s_bf[:, ks, 0:JS].unsqueeze(2).broadcast_to([128, w, 2, JS]),
                s_yr[:, ks, :, :],
                mybir.AluOpType.mult,
            )

        bs0 = bsq_stripe(B_STRIPES[0])
        ps_a = psq_stripe(P_STRIPES[0])
        bexp_stripe(bs0)
        bs1 = bsq_stripe(B_STRIPES[1])
        pexp_stripe(ps_a)
        mul_stripe(bs0)
        ps_b = psq_stripe(P_STRIPES[1])
        bexp_stripe(bs1)
        mul_stripe(bs1)
        bs2 = bsq_stripe(B_STRIPES[2])
        pexp_stripe(ps_b)
        bexp_stripe(bs2)
        mul_stripe(bs2)
        bs3 = bsq_stripe(B_STRIPES[3])
        bexp_stripe(bs3)
        mul_stripe(bs3)
        assert p_done == NCHUNK and b_done == NCHUNK

        # ---- contraction: M[k, (c,j)] = sum_x P[k,x] * BfT[x,(c,j)] ----
        accm = psum.tile([NANCH, 3 * JS], f32, tag="accm")
        for k in range(NCHUNK):
            nc.tensor.matmul(
                accm[:, :],
                s_p[:, k, :],        # stationary lhsT: [128, 12] fp16
                s_bf[:, k, :],       # moving rhs: [128, 48] fp16 contiguous
                start=(k == 0),
                stop=(k == NCHUNK - 1),
            )

        # ---- decompress: acc[i, (c,j)] = sum_k F[k,i] * M[k,(c,j)] ----
        s_m = singles.tile([NANCH, 3 * JS], f16, tag="m")
        nc.vector.tensor_copy(s_m[:, :], accm[:, :])
        acc = psum.tile([128, 3 * JS], f32, tag="acc")
        nc.tensor.matmul(
            acc[:, :],
            s_fc[:, :],              # stationary lhsT: [12, 128] fp16
            s_m[:, :],               # moving rhs: [12, 48] fp16
            start=True,
            stop=True,
        )

        # ---- epilogue: normalize columns 1,2 by column 0 (the row-sum) ----
        s_rec = singles.tile([128, JS], f32, tag="rec")
        nc.vector.reciprocal_approx_fast(s_rec[:, :], acc[:, 0:JS])
        s_out = singles.tile([128, 3 * JS], f32, tag="outt")
        nc.scalar.copy(s_out[:, 0:JS], acc[:, 0:JS])
        nc.vector.tensor_tensor(
            s_out[:, JS:3 * JS].rearrange("p (c j) -> p c j", c=2),
            acc[:, JS:3 * JS].rearrange("p (c j) -> p c j", c=2),
            s_rec[:, :].unsqueeze(1).broadcast_to([128, 2, JS]),
            mybir.AluOpType.mult,
        )
        nc.sync.dma_start(out[:, :], s_out[:, :])

    nc.finalize()
    return nc


def _get_program():
    if "nc" not in _CACHE:
        _CACHE["nc"] = _build_program()
    return _CACHE["nc"]


def _host_inputs(X, Y):
    """Build the per-core input maps (layout prep + host constants only)."""
    X = np.ascontiguousarray(np.asarray(X, dtype=np.float32))
    Y = np.ascontiguousarray(np.asarray(Y, dtype=np.float32))
    ys = np.linspace(GRID_LO, GRID_HI, N_AXIS, dtype=np.float32)
    a, F = _anchors_and_weights()

    ac = np.empty((128, 64 + NANCH), np.float32)
    ac[:, 0:64] = X[:, 0].reshape(NCHUNK, 128).T
    ac[:, 64:64 + NANCH] = a[None, :]
    # yr[p, ck, c, j] = Y[:, c] chunk-transposed, broadcast over j
    yt = np.stack([Y[:, 0].reshape(NCHUNK, 128).T,
                   Y[:, 1].reshape(NCHUNK, 128).T], axis=2)  # [128, ck, 2]
    yr = np.broadcast_to(yt[:, :, :, None],
                         (128, NCHUNK, 2, JS)).reshape(128, -1)
    yr = np.ascontiguousarray(yr, dtype=np.float16)

    x1t = X[:, 1].reshape(NCHUNK, 128).T
    in_maps = []
    for m in range(NCORES):
        bcm = np.empty((128, 80), np.float32)
        bcm[:, 0:64] = x1t
        bcm[:, 64:80] = ys[m * JS:(m + 1) * JS][None, :]
        in_maps.append({"bc": bcm, "ac": ac, "yr": yr,
                        "fc": F.astype(np.float16)})
    return in_maps


def run_on_cores(X, Y, **spmd_kwargs):
    """Run the SPMD kernel; returns BassKernelResults."""
    from concourse.bass_utils import run_bass_kernel_spmd

    nc = _get_program()
    in_maps = _host_inputs(X, Y)
    res = run_bass_kernel_spmd(nc, in_maps, core_ids=list(range(NCORES)),
                               **spmd_kwargs)
    return res


def kernel(X, Y):
    res = run_on_cores(X, Y)
    full = np.empty((3, N_AXIS, N_AXIS), dtype=np.float32)
    for m, r in enumerate(res.results):
        blk = r["out"]  # [128, 48] rows = i, cols = (c, j_local)
        for c in range(3):
            full[c, m * JS:(m + 1) * JS, :] = blk[:, c * JS:(c + 1) * JS].T
    return full
